# revision 1
# baseline (speedup 1.0000x reference)
"""NonLocalAttention Trainium2 kernel.

Math per batch b (reference):
  q/k/v = conv1x1(x, w*, b*)            # [CI, N], N = H*W = 4096, CI = 128
  attn  = softmax(q^T k, axis=-1)       # [N, N]
  o     = v @ attn^T                    # [CI, N]
  out   = gamma * (wo @ o + bo) + x     # [C, N]

Distribution: data-parallel over batch, one batch per NeuronCore (B = 8 = n_cores).

Per-core algorithm (all big matmuls in bf16, fp32 PSUM accumulation):
  - Q, K        = wT.T @ x  (+bias)              [CI=128 part, N free]
  - V^T chunks  = x_chunk.T @ wvT                [128 j-rows, CI]  (bias folded out, see below)
  - S^T[j, i]   = K_chunk.T @ Q  computed directly with j on partitions -> no transposes
  - A_u         = exp(S^T) on ScalarE (no max subtraction needed: logits are O(±8), fp32 exp safe)
  - O_u[c, i]   = sum_jc VT_chunk.T @ A_u_chunk  (PSUM accumulation over 32 chunks)
  - sums[*, i]  = sum_jc ones.T @ A_u_chunk      (softmax denominators via PE matvec)
  - O = O_u * (1/sums); out = gamma*(woT.T @ O) + gbo + x
  - softmax row-normalization commutes with the V and wo matmuls; the V-bias term
    contributes bv[c] * sum_j A[i,j]/sums[i] = bv[c], so it folds into a host-side
    constant gbo = gamma*(wo@bv + bo).
"""

import numpy as np
import ml_dtypes

B, C = 8, 256
HH, WW = 64, 64
N = HH * WW          # 4096
CI = 128
P = 128
IB = 1024            # i-block (columns of S^T processed per PSUM round)
NIB = N // IB        # 4
NJC = N // P         # 32 j-chunks
FD = 512             # matmul moving-operand free dim (one PSUM bank fp32)
NCORES = 8

_CACHE = {}


def _build():
    if "nc" in _CACHE:
        return _CACHE["nc"]
    from contextlib import ExitStack
    import concourse.bacc as bacc
    import concourse.tile as tile
    from concourse import mybir

    f32 = mybir.dt.float32
    bf16 = mybir.dt.bfloat16
    EXP = mybir.ActivationFunctionType.Exp

    nc = bacc.Bacc("TRN2", target_bir_lowering=False, debug=False, num_devices=NCORES)

    x_f = nc.dram_tensor("x_f", [2, P, N], f32, kind="ExternalInput").ap()
    x_b = nc.dram_tensor("x_b", [2, P, N], bf16, kind="ExternalInput").ap()
    wqT_d = nc.dram_tensor("wqT", [2, P, CI], bf16, kind="ExternalInput").ap()
    wkT_d = nc.dram_tensor("wkT", [2, P, CI], bf16, kind="ExternalInput").ap()
    wvT_d = nc.dram_tensor("wvT", [2, P, CI], bf16, kind="ExternalInput").ap()
    woT_d = nc.dram_tensor("woT", [P, C], bf16, kind="ExternalInput").ap()
    bq_d = nc.dram_tensor("bq", [P, 1], f32, kind="ExternalInput").ap()
    bk_d = nc.dram_tensor("bk", [P, 1], f32, kind="ExternalInput").ap()
    gbo_d = nc.dram_tensor("gbo", [P, 2], f32, kind="ExternalInput").ap()
    gam_d = nc.dram_tensor("gam", [P, 1], f32, kind="ExternalInput").ap()
    out_d = nc.dram_tensor("out", [C, N], f32, kind="ExternalOutput").ap()

    with tile.TileContext(nc) as tc, ExitStack() as ctx:
        sb = ctx.enter_context(tc.tile_pool(name="sb", bufs=1))
        wk_pool = ctx.enter_context(tc.tile_pool(name="wk", bufs=1))
        ps = ctx.enter_context(tc.tile_pool(name="ps", bufs=1, space="PSUM"))

        # ---- persistent SBUF tensors ----
        Xf = [sb.tile([P, N], f32, name=f"Xf{c}") for c in range(2)]
        Xb = [sb.tile([P, N], bf16, name=f"Xb{c}") for c in range(2)]
        Qs = sb.tile([P, N], bf16, name="Qs")
        Ks = sb.tile([P, N], bf16, name="Ks")
        VT = sb.tile([P, N], bf16, name="VT")
        wqT_s = sb.tile([P, C], bf16, name="wqT_s")
        wkT_s = sb.tile([P, C], bf16, name="wkT_s")
        wvT_s = sb.tile([P, C], bf16, name="wvT_s")
        woT_s = sb.tile([P, C], bf16, name="woT_s")
        bq_s = sb.tile([P, 1], f32, name="bq_s")
        bk_s = sb.tile([P, 1], f32, name="bk_s")
        gbo_s = sb.tile([P, 2], f32, name="gbo_s")
        gam_s = sb.tile([P, 1], f32, name="gam_s")
        ones_s = sb.tile([P, P], bf16, name="ones_s")

        # ---- input DMAs ----
        for cc in range(2):
            nc.sync.dma_start(out=wqT_s[:, cc * CI:(cc + 1) * CI], in_=wqT_d[cc])
            nc.sync.dma_start(out=wkT_s[:, cc * CI:(cc + 1) * CI], in_=wkT_d[cc])
            nc.sync.dma_start(out=wvT_s[:, cc * CI:(cc + 1) * CI], in_=wvT_d[cc])
        nc.sync.dma_start(out=woT_s, in_=woT_d)
        nc.sync.dma_start(out=bq_s, in_=bq_d)
        nc.sync.dma_start(out=bk_s, in_=bk_d)
        nc.sync.dma_start(out=gbo_s, in_=gbo_d)
        nc.sync.dma_start(out=gam_s, in_=gam_d)
        nc.vector.memset(ones_s, 1.0)
        for cc in range(2):
            for q in range(4):
                sl = slice(q * 1024, (q + 1) * 1024)
                nc.sync.dma_start(out=Xb[cc][:, sl], in_=x_b[cc, :, sl])
        for cc in range(2):
            for q in range(4):
                sl = slice(q * 1024, (q + 1) * 1024)
                nc.sync.dma_start(out=Xf[cc][:, sl], in_=x_f[cc, :, sl])

        # ---- Q, K projections: [CI, N] bf16, bias added on DVE during PSUM->SBUF ----
        for wname, W_s, b_s, OUT in (("q", wqT_s, bq_s, Qs), ("k", wkT_s, bk_s, Ks)):
            for s8 in range(N // FD):
                sl = slice(s8 * FD, (s8 + 1) * FD)
                pj = ps.tile([P, FD], f32, tag="st", bufs=2, name=f"p{wname}{s8}")
                for cc in range(2):
                    nc.tensor.matmul(
                        pj, lhsT=W_s[:, cc * CI:(cc + 1) * CI], rhs=Xb[cc][:, sl],
                        start=(cc == 0), stop=(cc == 1))
                nc.vector.tensor_scalar_add(out=OUT[:, sl], in0=pj, scalar1=b_s)

        # ---- V^T: chunk jc is [128 rows of n, CI] at VT[:, jc*128:(jc+1)*128] ----
        for jc in range(NJC):
            slj = slice(jc * P, (jc + 1) * P)
            pv = ps.tile([P, P], f32, tag="st", bufs=2, name=f"pv{jc}")
            for cc in range(2):
                nc.tensor.matmul(
                    pv, lhsT=Xb[cc][:, slj], rhs=wvT_s[:, cc * CI:(cc + 1) * CI],
                    start=(cc == 0), stop=(cc == 1))
            nc.vector.tensor_copy(out=VT[:, slj], in_=pv)

        # ---- attention main loop ----
        def do_st(ib, jc):
            """S^T chunk [j=128, i=IB] -> exp -> bf16 SBUF."""
            i0 = ib * IB
            st_ps = ps.tile([P, IB], f32, tag="st", bufs=2, name=f"st{ib}_{jc}")
            for h in range(IB // FD):
                sl = slice(h * FD, (h + 1) * FD)
                nc.tensor.matmul(
                    st_ps[:, sl],
                    lhsT=Ks[:, jc * P:(jc + 1) * P],
                    rhs=Qs[:, i0 + h * FD: i0 + (h + 1) * FD],
                    start=True, stop=True)
            a_sb = wk_pool.tile([P, IB], bf16, tag="a", bufs=3, name=f"a{ib}_{jc}")
            nc.scalar.activation(a_sb, st_ps, EXP)
            return a_sb

        for ib in range(NIB):
            i0 = ib * IB
            o_ps = ps.tile([P, IB], f32, tag="o", bufs=1, name=f"o{ib}")
            s_ps = ps.tile([P, IB], f32, tag="sums", bufs=1, name=f"s{ib}")
            a_cur = do_st(ib, 0)
            for jc in range(NJC):
                a_next = do_st(ib, jc + 1) if jc + 1 < NJC else None
                for h in range(IB // FD):
                    sl = slice(h * FD, (h + 1) * FD)
                    nc.tensor.matmul(
                        o_ps[:, sl], lhsT=VT[:, jc * P:(jc + 1) * P], rhs=a_cur[:, sl],
                        start=(jc == 0), stop=(jc == NJC - 1))
                    nc.tensor.matmul(
                        s_ps[:, sl], lhsT=ones_s, rhs=a_cur[:, sl],
                        start=(jc == 0), stop=(jc == NJC - 1))
                a_cur = a_next

            rec = wk_pool.tile([P, IB], f32, tag="rec", bufs=2, name=f"rec{ib}")
            nc.vector.reciprocal(rec, s_ps)
            onorm = wk_pool.tile([P, IB], bf16, tag="onorm", bufs=2, name=f"on{ib}")
            nc.vector.tensor_mul(onorm, o_ps, rec)

            # ---- output projection + residual for this i-block ----
            for ch in range(2):
                z_ps = ps.tile([P, IB], f32, tag="st", bufs=2, name=f"z{ib}_{ch}")
                for h in range(IB // FD):
                    sl = slice(h * FD, (h + 1) * FD)
                    nc.tensor.matmul(
                        z_ps[:, sl], lhsT=woT_s[:, ch * CI:(ch + 1) * CI],
                        rhs=onorm[:, sl], start=True, stop=True)
                y_sb = wk_pool.tile([P, IB], f32, tag="y", bufs=2, name=f"y{ib}_{ch}")
                # y = gamma*z + gbo[ch]
                nc.vector.tensor_scalar(
                    out=y_sb, in0=z_ps, scalar1=gam_s, scalar2=gbo_s[:, ch:ch + 1],
                    op0=mybir.AluOpType.mult, op1=mybir.AluOpType.add)
                nc.vector.tensor_add(y_sb, y_sb, Xf[ch][:, i0:i0 + IB])
                nc.sync.dma_start(
                    out=out_d[ch * P:(ch + 1) * P, i0:i0 + IB], in_=y_sb)

    nc.compile()
    _CACHE["nc"] = nc
    return nc


def _in_maps(x, wq, bq, wk, bk, wv, bv, wo, bo, gamma):
    bf = ml_dtypes.bfloat16
    x = np.asarray(x, np.float32).reshape(B, 2, P, N)
    wq = np.asarray(wq, np.float32)
    wk = np.asarray(wk, np.float32)
    wv = np.asarray(wv, np.float32)
    wo = np.asarray(wo, np.float32)
    bq = np.asarray(bq, np.float32)
    bk = np.asarray(bk, np.float32)
    bv = np.asarray(bv, np.float32)
    bo = np.asarray(bo, np.float32)
    g = float(np.asarray(gamma, np.float32)[0])

    wqT = np.ascontiguousarray(wq.T).reshape(2, P, CI).astype(bf)
    wkT = np.ascontiguousarray(wk.T).reshape(2, P, CI).astype(bf)
    wvT = np.ascontiguousarray(wv.T).reshape(2, P, CI).astype(bf)
    woT = np.ascontiguousarray(wo.T).astype(bf)                     # [CI, C]
    gbo = np.ascontiguousarray((g * (wo @ bv + bo)).reshape(2, P).T).astype(np.float32)
    gam = np.full((P, 1), g, np.float32)
    bq2 = np.ascontiguousarray(bq.reshape(P, 1))
    bk2 = np.ascontiguousarray(bk.reshape(P, 1))

    maps = []
    for b in range(B):
        xb = np.ascontiguousarray(x[b])
        maps.append(dict(
            x_f=xb, x_b=xb.astype(bf), wqT=wqT, wkT=wkT, wvT=wvT, woT=woT,
            bq=bq2, bk=bk2, gbo=gbo, gam=gam))
    return maps


def run(trace=False, **inputs):
    import concourse.bass_utils as bass_utils
    nc = _build()
    maps = _in_maps(**inputs)
    res = bass_utils.run_bass_kernel_spmd(
        nc, maps, core_ids=list(range(NCORES)), trace=trace)
    out = np.stack([r["out"] for r in res.results])
    return out.reshape(B, C, HH, WW).astype(np.float32), res


def kernel(**inputs):
    out, _ = run(trace=False, **inputs)
    return out


# revision 4
# speedup vs baseline: 14807.7483x; 14807.7483x over previous
"""NonLocalAttention Trainium2 kernel.

Math per batch b (reference):
  q/k/v = conv1x1(x, w*, b*)            # [CI, N], N = H*W = 4096, CI = 128
  attn  = softmax(q^T k, axis=-1)       # [N, N]
  o     = v @ attn^T                    # [CI, N]
  out   = gamma * (wo @ o + bo) + x     # [C, N]

Distribution: data-parallel over batch, one batch per NeuronCore (B = 8 = n_cores).

Per-core algorithm (all big matmuls in bf16, fp32 PSUM accumulation):
  - Q, K        = wT.T @ x  (+bias)              [CI=128 part, N free]
  - V^T chunks  = x_chunk.T @ wvT                [128 j-rows, CI]  (bias folded out, see below)
  - S^T[j, i]   = K_chunk.T @ Q  computed directly with j on partitions -> no transposes
  - A_u         = exp(S^T) on ScalarE (no max subtraction needed: logits are O(±8), fp32 exp safe)
  - O_u[c, i]   = sum_jc VT_chunk.T @ A_u_chunk  (PSUM accumulation over 32 chunks)
  - sums[*, i]  = sum_jc ones.T @ A_u_chunk      (softmax denominators via PE matvec)
  - O = O_u * (1/sums); out = gamma*(woT.T @ O) + gbo + x
  - softmax row-normalization commutes with the V and wo matmuls; the V-bias term
    contributes bv[c] * sum_j A[i,j]/sums[i] = bv[c], so it folds into a host-side
    constant gbo = gamma*(wo@bv + bo).
"""

import numpy as np
import ml_dtypes

B, C = 8, 256
HH, WW = 64, 64
N = HH * WW          # 4096
CI = 128
P = 128
IB = 1024            # i-block (columns of S^T processed per PSUM round)
NIB = N // IB        # 4
NJC = N // P         # 32 j-chunks
FD = 512             # matmul moving-operand free dim (one PSUM bank fp32)
NCORES = 8

_CACHE = {}


def _build(reps=1):
    key = ("nc", reps)
    if key in _CACHE:
        return _CACHE[key]
    from contextlib import ExitStack
    import concourse.bacc as bacc
    import concourse.tile as tile
    from concourse import mybir

    f32 = mybir.dt.float32
    bf16 = mybir.dt.bfloat16
    EXP = mybir.ActivationFunctionType.Exp

    nc = bacc.Bacc("TRN2", target_bir_lowering=False, debug=False, num_devices=NCORES)

    x_f = nc.dram_tensor("x_f", [2, P, N], f32, kind="ExternalInput").ap()
    x_b = nc.dram_tensor("x_b", [2, P, N], bf16, kind="ExternalInput").ap()
    wqT_d = nc.dram_tensor("wqT", [2, P, CI], bf16, kind="ExternalInput").ap()
    wkT_d = nc.dram_tensor("wkT", [2, P, CI], bf16, kind="ExternalInput").ap()
    wvT_d = nc.dram_tensor("wvT", [2, P, CI], bf16, kind="ExternalInput").ap()
    woT_d = nc.dram_tensor("woT", [P, C], bf16, kind="ExternalInput").ap()
    bq_d = nc.dram_tensor("bq", [P, 1], f32, kind="ExternalInput").ap()
    bk_d = nc.dram_tensor("bk", [P, 1], f32, kind="ExternalInput").ap()
    gbo_d = nc.dram_tensor("gbo", [P, 2], f32, kind="ExternalInput").ap()
    gam_d = nc.dram_tensor("gam", [P, 1], f32, kind="ExternalInput").ap()
    out_d = nc.dram_tensor("out", [C, N], f32, kind="ExternalOutput").ap()

    with tile.TileContext(nc) as tc, ExitStack() as ctx:
        sb = ctx.enter_context(tc.tile_pool(name="sb", bufs=1))
        wk_pool = ctx.enter_context(tc.tile_pool(name="wk", bufs=1))
        ps = ctx.enter_context(tc.tile_pool(name="ps", bufs=1, space="PSUM"))

        # ---- persistent SBUF tensors ----
        Xf = [sb.tile([P, N], f32, name=f"Xf{c}") for c in range(2)]
        Xb = [sb.tile([P, N], bf16, name=f"Xb{c}") for c in range(2)]
        Qs = sb.tile([P, N], bf16, name="Qs")
        Ks = sb.tile([P, N], bf16, name="Ks")
        VT = sb.tile([P, N], bf16, name="VT")
        wqT_s = sb.tile([P, C], bf16, name="wqT_s")
        wkT_s = sb.tile([P, C], bf16, name="wkT_s")
        wvT_s = sb.tile([P, C], bf16, name="wvT_s")
        woT_s = sb.tile([P, C], bf16, name="woT_s")
        bq_s = sb.tile([P, 1], f32, name="bq_s")
        bk_s = sb.tile([P, 1], f32, name="bk_s")
        gbo_s = sb.tile([P, 2], f32, name="gbo_s")
        gam_s = sb.tile([P, 1], f32, name="gam_s")
        ones_s = sb.tile([P, P], bf16, name="ones_s")

        # ---- input DMAs ----
        for cc in range(2):
            nc.sync.dma_start(out=wqT_s[:, cc * CI:(cc + 1) * CI], in_=wqT_d[cc])
            nc.sync.dma_start(out=wkT_s[:, cc * CI:(cc + 1) * CI], in_=wkT_d[cc])
            nc.sync.dma_start(out=wvT_s[:, cc * CI:(cc + 1) * CI], in_=wvT_d[cc])
        nc.sync.dma_start(out=woT_s, in_=woT_d)
        nc.sync.dma_start(out=bq_s, in_=bq_d)
        nc.sync.dma_start(out=bk_s, in_=bk_d)
        nc.sync.dma_start(out=gbo_s, in_=gbo_d)
        nc.sync.dma_start(out=gam_s, in_=gam_d)
        nc.vector.memset(ones_s, 1.0)
        for cc in range(2):
            for q in range(4):
                sl = slice(q * 1024, (q + 1) * 1024)
                nc.sync.dma_start(out=Xb[cc][:, sl], in_=x_b[cc, :, sl])
        for cc in range(2):
            for q in range(4):
                sl = slice(q * 1024, (q + 1) * 1024)
                nc.sync.dma_start(out=Xf[cc][:, sl], in_=x_f[cc, :, sl])

        # ---- Q, K projections: [CI, N] bf16, bias added on DVE during PSUM->SBUF ----
        for wname, W_s, b_s, OUT in (("q", wqT_s, bq_s, Qs), ("k", wkT_s, bk_s, Ks)):
            for s8 in range(N // FD):
                sl = slice(s8 * FD, (s8 + 1) * FD)
                pj = ps.tile([P, FD], f32, tag="st", bufs=2, name=f"p{wname}{s8}")
                for cc in range(2):
                    nc.tensor.matmul(
                        pj, lhsT=W_s[:, cc * CI:(cc + 1) * CI], rhs=Xb[cc][:, sl],
                        start=(cc == 0), stop=(cc == 1))
                nc.vector.tensor_scalar_add(out=OUT[:, sl], in0=pj, scalar1=b_s)

        # ---- V^T: chunk jc is [128 rows of n, CI] at VT[:, jc*128:(jc+1)*128] ----
        for jc in range(NJC):
            slj = slice(jc * P, (jc + 1) * P)
            pv = ps.tile([P, P], f32, tag="st", bufs=2, name=f"pv{jc}")
            for cc in range(2):
                nc.tensor.matmul(
                    pv, lhsT=Xb[cc][:, slj], rhs=wvT_s[:, cc * CI:(cc + 1) * CI],
                    start=(cc == 0), stop=(cc == 1))
            nc.vector.tensor_copy(out=VT[:, slj], in_=pv)

        # ---- attention main loop ----
        for _rep in range(reps):
            _main(nc, tc, ps, wk_pool, mybir, f32, bf16, EXP,
                  Xf, Qs, Ks, VT, woT_s, gbo_s, gam_s, ones_s, out_d)

    nc.compile()
    _CACHE[key] = nc
    return nc


def _main(nc, tc, ps, wk_pool, mybir, f32, bf16, EXP,
          Xf, Qs, Ks, VT, woT_s, gbo_s, gam_s, ones_s, out_d):
    if True:
        def do_st(ib, jc):
            """S^T chunk [j=128, i=IB] -> exp -> bf16 SBUF."""
            i0 = ib * IB
            st_ps = ps.tile([P, IB], f32, tag="st", bufs=2, name=f"st{ib}_{jc}")
            for h in range(IB // FD):
                sl = slice(h * FD, (h + 1) * FD)
                nc.tensor.matmul(
                    st_ps[:, sl],
                    lhsT=Ks[:, jc * P:(jc + 1) * P],
                    rhs=Qs[:, i0 + h * FD: i0 + (h + 1) * FD],
                    start=True, stop=True)
            a_sb = wk_pool.tile([P, IB], bf16, tag="a", bufs=4, name=f"a{ib}_{jc}")
            nc.scalar.activation(a_sb, st_ps, EXP)
            return a_sb

        prefetched = []  # next ib's first S^T chunks, emitted before this ib's tail
        for ib in range(NIB):
            i0 = ib * IB
            o_ps = ps.tile([P, IB], f32, tag="o", bufs=1, name=f"o{ib}")
            s_ps = ps.tile([P, IB], f32, tag="sums", bufs=1, name=f"s{ib}")
            pre, prefetched = prefetched, []
            a_cur = pre.pop(0) if pre else do_st(ib, 0)
            for jc in range(NJC):
                if jc + 1 < NJC:
                    a_next = pre.pop(0) if pre else do_st(ib, jc + 1)
                else:
                    a_next = None
                    if ib + 1 < NIB:
                        # keep PE fed through the tail (recip/mul on DVE)
                        prefetched = [do_st(ib + 1, 0), do_st(ib + 1, 1)]
                for h in range(IB // FD):
                    sl = slice(h * FD, (h + 1) * FD)
                    nc.tensor.matmul(
                        o_ps[:, sl], lhsT=VT[:, jc * P:(jc + 1) * P], rhs=a_cur[:, sl],
                        start=(jc == 0), stop=(jc == NJC - 1))
                    nc.tensor.matmul(
                        s_ps[:, sl], lhsT=ones_s, rhs=a_cur[:, sl],
                        start=(jc == 0), stop=(jc == NJC - 1))
                a_cur = a_next

            rec = wk_pool.tile([P, IB], f32, tag="rec", bufs=2, name=f"rec{ib}")
            nc.vector.reciprocal(rec, s_ps)
            onorm = wk_pool.tile([P, IB], bf16, tag="onorm", bufs=2, name=f"on{ib}")
            nc.vector.tensor_mul(onorm, o_ps, rec)

            # ---- output projection + residual for this i-block ----
            for ch in range(2):
                z_ps = ps.tile([P, IB], f32, tag="st", bufs=2, name=f"z{ib}_{ch}")
                for h in range(IB // FD):
                    sl = slice(h * FD, (h + 1) * FD)
                    nc.tensor.matmul(
                        z_ps[:, sl], lhsT=woT_s[:, ch * CI:(ch + 1) * CI],
                        rhs=onorm[:, sl], start=True, stop=True)
                y_sb = wk_pool.tile([P, IB], f32, tag="y", bufs=2, name=f"y{ib}_{ch}")
                # y = gamma*z + gbo[ch]
                nc.vector.tensor_scalar(
                    out=y_sb, in0=z_ps, scalar1=gam_s, scalar2=gbo_s[:, ch:ch + 1],
                    op0=mybir.AluOpType.mult, op1=mybir.AluOpType.add)
                nc.vector.tensor_add(y_sb, y_sb, Xf[ch][:, i0:i0 + IB])
                nc.sync.dma_start(
                    out=out_d[ch * P:(ch + 1) * P, i0:i0 + IB], in_=y_sb)

def _in_maps(x, wq, bq, wk, bk, wv, bv, wo, bo, gamma):
    bf = ml_dtypes.bfloat16
    x = np.asarray(x, np.float32).reshape(B, 2, P, N)
    wq = np.asarray(wq, np.float32)
    wk = np.asarray(wk, np.float32)
    wv = np.asarray(wv, np.float32)
    wo = np.asarray(wo, np.float32)
    bq = np.asarray(bq, np.float32)
    bk = np.asarray(bk, np.float32)
    bv = np.asarray(bv, np.float32)
    bo = np.asarray(bo, np.float32)
    g = float(np.asarray(gamma, np.float32)[0])

    wqT = np.ascontiguousarray(wq.T).reshape(2, P, CI).astype(bf)
    wkT = np.ascontiguousarray(wk.T).reshape(2, P, CI).astype(bf)
    wvT = np.ascontiguousarray(wv.T).reshape(2, P, CI).astype(bf)
    woT = np.ascontiguousarray(wo.T).astype(bf)                     # [CI, C]
    gbo = np.ascontiguousarray((g * (wo @ bv + bo)).reshape(2, P).T).astype(np.float32)
    gam = np.full((P, 1), g, np.float32)
    bq2 = np.ascontiguousarray(bq.reshape(P, 1))
    bk2 = np.ascontiguousarray(bk.reshape(P, 1))

    maps = []
    for b in range(B):
        xb = np.ascontiguousarray(x[b])
        maps.append(dict(
            x_f=xb, x_b=xb.astype(bf), wqT=wqT, wkT=wkT, wvT=wvT, woT=woT,
            bq=bq2, bk=bk2, gbo=gbo, gam=gam))
    return maps


def run(trace=False, **inputs):
    import concourse.bass_utils as bass_utils
    nc = _build()
    maps = _in_maps(**inputs)
    res = bass_utils.run_bass_kernel_spmd(
        nc, maps, core_ids=list(range(NCORES)), trace=trace)
    out = np.stack([r["out"] for r in res.results])
    return out.reshape(B, C, HH, WW).astype(np.float32), res


def kernel(**inputs):
    out, _ = run(trace=False, **inputs)
    return out
